# revision 21
# baseline (speedup 1.0000x reference)
"""Trainium2 Bass kernel for DCT-based 2x frequency-domain super-resolution.

Reference computation (per image X = x[b, c] of shape [64, 64]):
    out[b,c] = DH2[:64,:]^T @ (DH @ X @ DW^T * mask[c]) @ DW2[:64,:]
             = mask[c] * (U @ X @ U^T),   U = DH2[:64,:]^T @ DH  (128x64)
(the zero-padding of high frequencies means only the first 64 rows/cols of
the 128-point DCT matrices participate; H == W so the row/col operators are
transposes of each other).

Strategy (memory-bound: 268 MiB f32 out + 4.2 MiB bf16 in over 8 cores):
  * Data-parallel over batch: 2 batches = 512 images = 256 image pairs per
    core; the [1,C,1,1] mask is folded into the input on the host (exact —
    it is a per-channel scalar that commutes with the transforms).
  * Host packs each image pair vertically into a [128, 64] bf16 slab
    (partition p = pair_parity*64 + h), stored partition-major so every
    input DMA is per-partition contiguous.
  * mm1: two concurrent quadrant matmuls (tile_position (0,0)/(64,64))
    compute (U @ X)^T for both images, stacked [128, 128] in one PSUM tile
    (K=64 each, rhs = [Ut; Ut]).  Batched 8 pairs per 2-bank PSUM tile.
  * One DVE/ACT copy (alternating engines) casts St2 to bf16 in SBUF.
  * mm2: lhsT = St2 pair slab (K=128), rhs = blockdiag(V, V) [128, 256]
    yields both 128x128 output images side by side; 4 pairs per 2-bank
    PSUM tile, one alternating-engine copy to the output staging buffer.
  * Output staged in SBUF and written with 1 MiB per-partition-contiguous
    DMAs to a [128, img, 128] partition-major DRAM layout (host transposes
    back); input DMAs ride the gpsimd/SWDGE ring so output owns the HWDGE
    ring.  Group sizes are ramped small->large->small to shorten pipeline
    fill/drain.

Measured ~115 us on 8 cores for the full problem (HBM roofline ~106 us);
bf16 compute gives rel l2 error ~3.5e-3 vs the f32 reference.
"""

import os
import numpy as np
import ml_dtypes

import concourse.mybir as mybir
from concourse import bacc
from concourse.tile import TileContext
from concourse.bass_utils import run_bass_kernel_spmd

BF16 = ml_dtypes.bfloat16

# Problem geometry (hardcoded per spec).
B, C, H, W = 16, 256, 64, 64
H2, W2 = 2 * H, 2 * W
N_CORES = 8
B_PER_CORE = B // N_CORES            # 2
IMGS = B_PER_CORE * C                # 512 images per core
PAIRS = IMGS // 2                    # 256 pairs per core

LAST_RESULT = None                   # BassKernelResults of the latest run


def _dct_mat(n):
    """Orthonormal DCT-II matrix in float64."""
    i = np.arange(n, dtype=np.float64)
    k = np.arange(n, dtype=np.float64)[:, None]
    m = np.cos(np.pi * (i + 0.5) * k / n)
    s = np.full((n, 1), np.sqrt(2.0 / n))
    s[0, 0] = np.sqrt(1.0 / n)
    return m * s


def _upsample_mat():
    """U = DH2[:64,:]^T @ DH, shape [128, 64]."""
    dh = _dct_mat(H)
    dh2 = _dct_mat(H2)
    return dh2[:H, :].T @ dh


def _make_nc():
    return bacc.Bacc(
        "TRN2",
        target_bir_lowering=False,
        debug=False,
        num_devices=N_CORES,
    )


# Tunable knobs (bench.py overrides these before building).
# Defaults = best measured config: vpair input (no zero padding), gpsimd-ring
# input DMAs, 8-pair (1 MiB) output DMAs with ramped group sizes, copies
# batched 4 pairs (st2: 8) and alternated across DVE/ACT.
CFG = dict(
    og_pairs=8,                 # pairs per output DMA (8 -> 1 MiB)
    ig_pairs=32,                # pairs per input DMA (32 -> 512 KiB vpair)
    in_engine="gpsimd",         # engine issuing input DMAs (SWDGE ring)
    out_engine="sync",          # engine issuing output DMAs (HWDGE ring)
    dma_only=False,             # skip compute; DMA in + DMA garbage out
    obuf_bufs=4,
    xin_bufs=4,
    mode="vpair",               # "blockdiag" (zero-padded pairs) or "vpair"
    cp_batch=4,                 # pairs per out-copy batch
    ps1_bufs=2,
    ps2_bufs=2,
    igs=[4, 4, 8, 16] + [32] * 7,              # input-group ramp (pairs)
    ogs=[2, 2, 2, 2] + [8] * 30 + [2, 2, 2, 2],  # output-group ramp (pairs)
    st_batch=8,                 # pairs per st2 PSUM tile/copy
    out_alt=False,              # alternate output DMAs across sync/scalar rings
    out_dtype="float16",        # DRAM output dtype (host upcasts to f32)
    cp_cycle="vs",              # copy-engine pattern: 'v'=DVE, 's'=ACT per copy
    direct_every=0,             # every Nth out-group: PSUM->DRAM casting DMA
    direct_phase=0,             #   (gpsimd SWDGE), skipping copy + staging
    cp_split=0.0,               # >0: split each copy DVE|ACT at this fraction
)

_ODT = {
    "float32": (mybir.dt.float32, np.float32),
    "float16": (mybir.dt.float16, np.float16),
    "bfloat16": (mybir.dt.bfloat16, BF16),
}


def _xin_shape():
    # blockdiag: [128, pair, 128] slab per pair; vpair: [128, pair, 64];
    # hpair: [64, pair, 128] (pair packed horizontally, h on partitions 0-63)
    if CFG["mode"] == "hpair":
        return [64, PAIRS, 128]
    return [128, PAIRS, 128 if CFG["mode"] == "blockdiag" else 64]


def _emit_body(nc, tc, xin, ut2, v2, out):
    """Emit one full pass over this core's 256 image pairs."""
    og_pairs = CFG["og_pairs"]
    ig_pairs = CFG["ig_pairs"]
    cpb = CFG["cp_batch"]                # pairs per PSUM->SBUF copy batch
    vpair = CFG["mode"] == "vpair"
    hpair = CFG["mode"] == "hpair"
    xw = 64 if vpair else 128            # free width per pair in xin
    xp = 64 if hpair else 128            # partitions in xin
    dma_in = getattr(nc, CFG["in_engine"])
    dma_out = getattr(nc, CFG["out_engine"])
    odt = _ODT[CFG["out_dtype"]][0]
    cyc = CFG["cp_cycle"]
    with (
        tc.tile_pool(name="const", bufs=1) as cpool,
        tc.tile_pool(name="xin", bufs=CFG["xin_bufs"]) as xpool,
        tc.tile_pool(name="st2", bufs=CFG.get("st2_bufs", 4)) as spool,
        tc.tile_pool(name="obuf", bufs=CFG["obuf_bufs"]) as opool,
        tc.tile_pool(name="ps1", bufs=CFG["ps1_bufs"], space="PSUM") as ps1,
        tc.tile_pool(name="ps2", bufs=CFG["ps2_bufs"], space="PSUM") as ps2,
    ):
        ut2_sb = cpool.tile([128, 128], mybir.dt.bfloat16)
        nc.sync.dma_start(out=ut2_sb[:], in_=ut2[:])
        v2_sb = cpool.tile([128, 256], mybir.dt.bfloat16)
        nc.sync.dma_start(out=v2_sb[:], in_=v2[:])

        ob_fixed = None
        if CFG["dma_only"]:
            ob_fixed = cpool.tile([128, og_pairs * 256], odt)
            nc.gpsimd.memset(ob_fixed[:], 0.0)

        igs = CFG["igs"] or [ig_pairs] * (PAIRS // ig_pairs)
        ogs = CFG["ogs"] or [og_pairs] * (PAIRS // og_pairs)
        assert sum(igs) == PAIRS and sum(ogs) == PAIRS, (igs, ogs)

        # pair index at which each input group starts -> its length
        ig_at = {}
        s = 0
        for L in igs:
            ig_at[s] = L
            s += L

        cur_xt, cur_base, qidx = None, 0, 0

        def ensure_input(pair):
            nonlocal cur_xt, cur_base
            if pair in ig_at:
                L = ig_at[pair]
                cur_xt = xpool.tile([xp, L * xw], mybir.dt.bfloat16)
                cur_base = pair
                src = xin[:, pair : pair + L, :]
                dma_in.dma_start(
                    out=cur_xt[:], in_=src.rearrange("p g f -> p (g f)")
                )
            return cur_xt, pair - cur_base

        cp_split = CFG["cp_split"]

        def copy_ps(dst, src):
            nonlocal qidx
            if cp_split:
                free = src.shape[-1]
                c = max(16, int(free * cp_split) // 16 * 16)
                nc.vector.tensor_copy(dst[:, :c], src[:, :c])
                nc.scalar.copy(dst[:, c:], src[:, c:])
            elif cyc[qidx % len(cyc)] == "v":
                nc.vector.tensor_copy(dst, src)
            else:
                nc.scalar.copy(dst, src)
            qidx += 1

        de, dp = CFG["direct_every"], CFG["direct_phase"]
        og_base = 0
        for g_idx, og_len in enumerate(ogs):
            direct = bool(de) and (g_idx % de == dp)
            ob = None
            if not direct:
                ob = ob_fixed if ob_fixed is not None else opool.tile(
                    [128, og_len * 256], odt
                )
            if CFG["dma_only"]:
                for p in range(og_len):
                    ensure_input(og_base + p)
            else:
                stb = CFG["st_batch"] or cpb
                off = 0
                while off < og_len:
                    sb_len = min(stb, og_len - off)
                    # --- mm1 for `sb_len` pairs into one PSUM tile ---
                    st2_ps = ps1.tile([128, sb_len * 128], mybir.dt.float32)
                    for p in range(sb_len):
                        xt, li = ensure_input(og_base + off + p)
                        fs = slice(p * 128, (p + 1) * 128)
                        if hpair:
                            # One matmul per pair: lhsT [64h, 128] holds both
                            # images' w-columns side by side -> st2 [128, 128]
                            # with even w on partitions 0-63, odd on 64-127.
                            nc.tensor.matmul(
                                st2_ps[:, fs],
                                lhsT=xt[0:64, li * 128 : (li + 1) * 128],
                                rhs=ut2_sb[0:64, :],
                                start=True,
                                stop=True,
                            )
                        elif vpair:
                            # Concurrent quadrant matmuls: even image in
                            # rows/cols 0-63, odd in rows/cols 64-127.
                            nc.tensor.matmul(
                                st2_ps[0:64, fs],
                                lhsT=xt[0:64, li * 64 : (li + 1) * 64],
                                rhs=ut2_sb[0:64, :],
                                start=True,
                                stop=True,
                                tile_position=(0, 0),
                            )
                            nc.tensor.matmul(
                                st2_ps[64:128, fs],
                                lhsT=xt[64:128, li * 64 : (li + 1) * 64],
                                rhs=ut2_sb[64:128, :],
                                start=True,
                                stop=True,
                                tile_position=(64, 64),
                            )
                        else:
                            nc.tensor.matmul(
                                st2_ps[:, fs],
                                lhsT=xt[:, li * 128 : (li + 1) * 128],
                                rhs=ut2_sb[:],
                                start=True,
                                stop=True,
                            )
                    # --- st2 copy (cast to bf16), engine per cp_cycle ---
                    st2_sb = spool.tile([128, sb_len * 128], mybir.dt.bfloat16)
                    copy_ps(st2_sb[:], st2_ps[:])
                    # --- mm2 + out copy in batches of cpb pairs ---
                    off2 = 0
                    while off2 < sb_len:
                        chunk = min(cpb, sb_len - off2)
                        o_ps = ps2.tile([128, chunk * 256], mybir.dt.float32)
                        for p in range(chunk):
                            nc.tensor.matmul(
                                o_ps[:, p * 256 : (p + 1) * 256],
                                lhsT=st2_sb[:, (off2 + p) * 128 : (off2 + p + 1) * 128],
                                rhs=v2_sb[:],
                                start=True,
                                stop=True,
                            )
                        if direct:
                            p0 = og_base + off + off2
                            dst = out[:, p0 * 2 : (p0 + chunk) * 2, :]
                            nc.gpsimd.dma_start(
                                out=dst.rearrange("p g f -> p (g f)"),
                                in_=o_ps[:],
                            )
                        else:
                            oslice = ob[
                                :, (off + off2) * 256 : (off + off2 + chunk) * 256
                            ]
                            copy_ps(oslice, o_ps[:])
                        off2 += chunk
                    off += sb_len
            if not direct:
                dst = out[:, og_base * 2 : (og_base + og_len) * 2, :]
                eng = dma_out
                if CFG["out_alt"]:
                    eng = (
                        nc.sync
                        if (og_base // max(og_len, 1)) % 2 == 0
                        else nc.scalar
                    )
                eng.dma_start(
                    out=dst.rearrange("p g f -> p (g f)"), in_=ob[:]
                )
            og_base += og_len


_NC_CACHE = None


def _build_nc():
    nc = _make_nc()
    xin = nc.declare_dram_parameter(
        "xin", _xin_shape(), mybir.dt.bfloat16, isOutput=False
    )
    ut2 = nc.declare_dram_parameter(
        "ut2", [128, 128], mybir.dt.bfloat16, isOutput=False
    )
    v2 = nc.declare_dram_parameter(
        "v2", [128, 256], mybir.dt.bfloat16, isOutput=False
    )
    out = nc.declare_dram_parameter(
        "out", [128, IMGS, 128], _ODT[CFG["out_dtype"]][0], isOutput=True
    )
    with TileContext(nc) as tc:
        _emit_body(nc, tc, xin, ut2, v2, out)
    nc.compile()
    return nc


def build_nc_timed(iters: int):
    """Benchmark variant: internal DRAM I/O, body repeated `iters` times
    via a device-side loop, tiny external output for minimal transfer."""
    nc = _make_nc()
    dummy_in = nc.declare_dram_parameter(
        "dummy_in", [1, 4], mybir.dt.float32, isOutput=False
    )
    dummy_out = nc.declare_dram_parameter(
        "dummy_out", [1, 4], mybir.dt.float32, isOutput=True
    )
    xin = nc.dram_tensor("xin_i", _xin_shape(), mybir.dt.bfloat16)
    ut2 = nc.dram_tensor("ut2_i", [128, 128], mybir.dt.bfloat16)
    v2 = nc.dram_tensor("v2_i", [128, 256], mybir.dt.bfloat16)
    out = nc.dram_tensor("out_i", [128, IMGS, 128], _ODT[CFG["out_dtype"]][0])
    with TileContext(nc) as tc:
        if iters == 1:
            _emit_body(nc, tc, xin, ut2, v2, out)
        else:
            with tc.For_i(0, iters, 1):
                _emit_body(nc, tc, xin, ut2, v2, out)
        with tc.tile_pool(name="dummy", bufs=1) as dpool:
            dt_sb = dpool.tile([1, 4], mybir.dt.float32)
            nc.sync.dma_start(out=dt_sb[:], in_=dummy_in[:])
            nc.sync.dma_start(out=dummy_out[:], in_=dt_sb[:])
    nc.compile()
    return nc


def _host_pack(x_lowres, sparse_mask):
    """Fold mask into input and pack per-core block-diagonal pair slabs."""
    u = _upsample_mat()                      # [128, 64] float64
    ut = u.T.astype(np.float32)              # [64, 128]
    ut2_np = np.concatenate([ut, ut], axis=0).astype(BF16)      # [128, 128]
    v2_np = np.zeros((128, 256), dtype=BF16)                    # blockdiag(V, V)
    v2_np[0:64, 0:128] = ut.astype(BF16)
    v2_np[64:128, 128:256] = ut.astype(BF16)

    xm = (x_lowres.astype(np.float32) * sparse_mask.astype(np.float32)).astype(BF16)

    mode = CFG["mode"]
    in_maps = []
    for i in range(N_CORES):
        imgs = xm[i * B_PER_CORE : (i + 1) * B_PER_CORE].reshape(IMGS, H, W)
        if mode == "hpair":
            xpack = np.empty((64, PAIRS, 128), dtype=BF16)
            xpack[:, :, 0:64] = imgs[0::2].transpose(1, 0, 2)
            xpack[:, :, 64:128] = imgs[1::2].transpose(1, 0, 2)
        elif mode == "vpair":
            xpack = np.empty((128, PAIRS, 64), dtype=BF16)
            xpack[0:64] = imgs[0::2].transpose(1, 0, 2)
            xpack[64:128] = imgs[1::2].transpose(1, 0, 2)
        else:
            xpack = np.zeros((128, PAIRS, 128), dtype=BF16)
            xpack[0:64, :, 0:64] = imgs[0::2].transpose(1, 0, 2)
            xpack[64:128, :, 64:128] = imgs[1::2].transpose(1, 0, 2)
        in_maps.append({"xin": xpack, "ut2": ut2_np, "v2": v2_np})
    return in_maps


def kernel(x_lowres: np.ndarray, sparse_mask: np.ndarray) -> np.ndarray:
    global _NC_CACHE, LAST_RESULT
    x_lowres = np.asarray(x_lowres)
    sparse_mask = np.asarray(sparse_mask)
    assert x_lowres.shape == (B, C, H, W), x_lowres.shape

    in_maps = _host_pack(x_lowres, sparse_mask)

    if _NC_CACHE is None:
        _NC_CACHE = _build_nc()
    nc = _NC_CACHE

    trace = bool(os.environ.get("BASS_TRACE"))
    try:
        res = run_bass_kernel_spmd(nc, in_maps, list(range(N_CORES)), trace=trace)
    except ModuleNotFoundError:
        # Trace path needs the axon NTFF hook; absent in slim containers.
        os.environ["BASS_NEVER_TRACE"] = "1"
        res = run_bass_kernel_spmd(nc, in_maps, list(range(N_CORES)), trace=False)
    LAST_RESULT = res

    out = np.empty((B, C, H2, W2), dtype=np.float32)
    for i in range(N_CORES):
        dev = np.asarray(res.results[i]["out"])          # [128, IMGS, 128]
        out[i * B_PER_CORE : (i + 1) * B_PER_CORE] = (
            dev.transpose(1, 0, 2).reshape(B_PER_CORE, C, H2, W2)
            .astype(np.float32)
        )
    return out



# revision 24
# speedup vs baseline: 1.1610x; 1.1610x over previous
"""Trainium2 Bass kernel for DCT-based 2x frequency-domain super-resolution.

Reference computation (per image X = x[b, c] of shape [64, 64]):
    out[b,c] = DH2[:64,:]^T @ (DH @ X @ DW^T * mask[c]) @ DW2[:64,:]
             = mask[c] * (U @ X @ U^T),   U = DH2[:64,:]^T @ DH  (128x64)
(the zero-padding of high frequencies means only the first 64 rows/cols of
the 128-point DCT matrices participate; H == W so the row/col operators are
transposes of each other).

Strategy (memory-bound; fp16 device output halves the dominant write
traffic: 134 MiB fp16 out + 4.2 MiB bf16 in per 8 cores, host upcasts):
  * Data-parallel over batch: 2 batches = 512 images = 256 image pairs per
    core; the [1,C,1,1] mask is folded into the input on the host (exact —
    it is a per-channel scalar that commutes with the transforms).
  * Host packs each image pair vertically into a [128, 64] bf16 slab
    (partition p = pair_parity*64 + h), stored partition-major so every
    input DMA is per-partition contiguous.
  * mm1: two concurrent quadrant matmuls (tile_position (0,0)/(64,64))
    compute (U @ X)^T for both images, stacked [128, 128] in one PSUM tile
    (K=64 each, rhs = [Ut; Ut]).  Batched 8 pairs per 2-bank PSUM tile.
  * One DVE/ACT copy (alternating engines) casts St2 to bf16 in SBUF.
  * mm2: lhsT = St2 pair slab (K=128), rhs = blockdiag(V, V) [128, 256]
    yields both 128x128 output images side by side; 4 pairs per 2-bank
    PSUM tile, one alternating-engine copy casts f32 PSUM to the fp16
    output staging buffer.
  * Output staged in SBUF and written with 0.5 MiB per-partition-contiguous
    DMAs to a [128, img, 128] partition-major fp16 DRAM layout (host
    transposes + upcasts); input DMAs ride the gpsimd/SWDGE ring so output
    owns the HWDGE ring.  Group sizes are ramped small->large->small to
    shorten pipeline fill/drain.

Measured ~73.7 us on 8 cores (dma-only floor of the same DMA pattern:
~67 us; f32-output version of this kernel: ~114.7 us).  bf16 compute +
fp16 output gives rel l2 error ~3.5e-3 vs the f32 reference (gate 2e-2).
"""

import os
import numpy as np
import ml_dtypes

import concourse.mybir as mybir
from concourse import bacc
from concourse.tile import TileContext
from concourse.bass_utils import run_bass_kernel_spmd

BF16 = ml_dtypes.bfloat16

# Problem geometry (hardcoded per spec).
B, C, H, W = 16, 256, 64, 64
H2, W2 = 2 * H, 2 * W
N_CORES = 8
B_PER_CORE = B // N_CORES            # 2
IMGS = B_PER_CORE * C                # 512 images per core
PAIRS = IMGS // 2                    # 256 pairs per core

LAST_RESULT = None                   # BassKernelResults of the latest run


def _dct_mat(n):
    """Orthonormal DCT-II matrix in float64."""
    i = np.arange(n, dtype=np.float64)
    k = np.arange(n, dtype=np.float64)[:, None]
    m = np.cos(np.pi * (i + 0.5) * k / n)
    s = np.full((n, 1), np.sqrt(2.0 / n))
    s[0, 0] = np.sqrt(1.0 / n)
    return m * s


def _upsample_mat():
    """U = DH2[:64,:]^T @ DH, shape [128, 64]."""
    dh = _dct_mat(H)
    dh2 = _dct_mat(H2)
    return dh2[:H, :].T @ dh


def _make_nc():
    return bacc.Bacc(
        "TRN2",
        target_bir_lowering=False,
        debug=False,
        num_devices=N_CORES,
    )


# Tunable knobs (bench.py overrides these before building).
# Defaults = best measured config: vpair input (no zero padding), gpsimd-ring
# input DMAs, 8-pair (1 MiB) output DMAs with ramped group sizes, copies
# batched 4 pairs (st2: 8) and alternated across DVE/ACT.
CFG = dict(
    og_pairs=8,                 # pairs per output DMA (8 -> 1 MiB)
    ig_pairs=32,                # pairs per input DMA (32 -> 512 KiB vpair)
    in_engine="gpsimd",         # engine issuing input DMAs (SWDGE ring)
    out_engine="sync",          # engine issuing output DMAs (HWDGE ring)
    dma_only=False,             # skip compute; DMA in + DMA garbage out
    obuf_bufs=8,
    xin_bufs=4,
    mode="vpair",               # "blockdiag" (zero-padded pairs) or "vpair"
    cp_batch=4,                 # pairs per out-copy batch
    ps1_bufs=2,
    ps2_bufs=2,
    igs=[4, 4, 8, 16] + [32] * 7,              # input-group ramp (pairs)
    ogs=[2, 2, 2, 2] + [8] * 30 + [2, 2, 2, 2],  # output-group ramp (pairs)
    st_batch=8,                 # pairs per st2 PSUM tile/copy
    out_alt=False,              # alternate output DMAs across sync/scalar rings
    out_dtype="float16",        # DRAM output dtype (host upcasts to f32)
    cp_cycle="vs",              # copy-engine pattern: 'v'=DVE, 's'=ACT per copy
    direct_every=0,             # every Nth out-group: PSUM->DRAM casting DMA
    direct_phase=0,             #   (gpsimd SWDGE), skipping copy + staging
    cp_split=0.0,               # >0: split each copy DVE|ACT at this fraction
)

_ODT = {
    "float32": (mybir.dt.float32, np.float32),
    "float16": (mybir.dt.float16, np.float16),
    "bfloat16": (mybir.dt.bfloat16, BF16),
}


def _xin_shape():
    # blockdiag: [128, pair, 128] slab per pair; vpair: [128, pair, 64];
    # hpair: [64, pair, 128] (pair packed horizontally, h on partitions 0-63)
    if CFG["mode"] == "hpair":
        return [64, PAIRS, 128]
    return [128, PAIRS, 128 if CFG["mode"] == "blockdiag" else 64]


def _emit_body(nc, tc, xin, ut2, v2, out):
    """Emit one full pass over this core's 256 image pairs."""
    og_pairs = CFG["og_pairs"]
    ig_pairs = CFG["ig_pairs"]
    cpb = CFG["cp_batch"]                # pairs per PSUM->SBUF copy batch
    vpair = CFG["mode"] == "vpair"
    hpair = CFG["mode"] == "hpair"
    xw = 64 if vpair else 128            # free width per pair in xin
    xp = 64 if hpair else 128            # partitions in xin
    dma_in = getattr(nc, CFG["in_engine"])
    dma_out = getattr(nc, CFG["out_engine"])
    odt = _ODT[CFG["out_dtype"]][0]
    cyc = CFG["cp_cycle"]
    with (
        tc.tile_pool(name="const", bufs=1) as cpool,
        tc.tile_pool(name="xin", bufs=CFG["xin_bufs"]) as xpool,
        tc.tile_pool(name="st2", bufs=CFG.get("st2_bufs", 4)) as spool,
        tc.tile_pool(name="obuf", bufs=CFG["obuf_bufs"]) as opool,
        tc.tile_pool(name="ps1", bufs=CFG["ps1_bufs"], space="PSUM") as ps1,
        tc.tile_pool(name="ps2", bufs=CFG["ps2_bufs"], space="PSUM") as ps2,
    ):
        ut2_sb = cpool.tile([128, 128], mybir.dt.bfloat16)
        nc.sync.dma_start(out=ut2_sb[:], in_=ut2[:])
        v2_sb = cpool.tile([128, 256], mybir.dt.bfloat16)
        nc.sync.dma_start(out=v2_sb[:], in_=v2[:])

        ob_fixed = None
        if CFG["dma_only"]:
            ob_fixed = cpool.tile([128, og_pairs * 256], odt)
            nc.gpsimd.memset(ob_fixed[:], 0.0)

        igs = CFG["igs"] or [ig_pairs] * (PAIRS // ig_pairs)
        ogs = CFG["ogs"] or [og_pairs] * (PAIRS // og_pairs)
        assert sum(igs) == PAIRS and sum(ogs) == PAIRS, (igs, ogs)

        # pair index at which each input group starts -> its length
        ig_at = {}
        s = 0
        for L in igs:
            ig_at[s] = L
            s += L

        cur_xt, cur_base, qidx = None, 0, 0

        def ensure_input(pair):
            nonlocal cur_xt, cur_base
            if pair in ig_at:
                L = ig_at[pair]
                cur_xt = xpool.tile([xp, L * xw], mybir.dt.bfloat16)
                cur_base = pair
                src = xin[:, pair : pair + L, :]
                dma_in.dma_start(
                    out=cur_xt[:], in_=src.rearrange("p g f -> p (g f)")
                )
            return cur_xt, pair - cur_base

        cp_split = CFG["cp_split"]

        def copy_ps(dst, src):
            nonlocal qidx
            if cp_split:
                free = src.shape[-1]
                c = max(16, int(free * cp_split) // 16 * 16)
                nc.vector.tensor_copy(dst[:, :c], src[:, :c])
                nc.scalar.copy(dst[:, c:], src[:, c:])
            elif cyc[qidx % len(cyc)] == "v":
                nc.vector.tensor_copy(dst, src)
            else:
                nc.scalar.copy(dst, src)
            qidx += 1

        de, dp = CFG["direct_every"], CFG["direct_phase"]
        og_base = 0
        for g_idx, og_len in enumerate(ogs):
            direct = bool(de) and (g_idx % de == dp)
            ob = None
            if not direct:
                ob = ob_fixed if ob_fixed is not None else opool.tile(
                    [128, og_len * 256], odt
                )
            if CFG["dma_only"]:
                for p in range(og_len):
                    ensure_input(og_base + p)
            else:
                stb = CFG["st_batch"] or cpb
                off = 0
                while off < og_len:
                    sb_len = min(stb, og_len - off)
                    # --- mm1 for `sb_len` pairs into one PSUM tile ---
                    st2_ps = ps1.tile([128, sb_len * 128], mybir.dt.float32)
                    for p in range(sb_len):
                        xt, li = ensure_input(og_base + off + p)
                        fs = slice(p * 128, (p + 1) * 128)
                        if hpair:
                            # One matmul per pair: lhsT [64h, 128] holds both
                            # images' w-columns side by side -> st2 [128, 128]
                            # with even w on partitions 0-63, odd on 64-127.
                            nc.tensor.matmul(
                                st2_ps[:, fs],
                                lhsT=xt[0:64, li * 128 : (li + 1) * 128],
                                rhs=ut2_sb[0:64, :],
                                start=True,
                                stop=True,
                            )
                        elif vpair:
                            # Concurrent quadrant matmuls: even image in
                            # rows/cols 0-63, odd in rows/cols 64-127.
                            nc.tensor.matmul(
                                st2_ps[0:64, fs],
                                lhsT=xt[0:64, li * 64 : (li + 1) * 64],
                                rhs=ut2_sb[0:64, :],
                                start=True,
                                stop=True,
                                tile_position=(0, 0),
                            )
                            nc.tensor.matmul(
                                st2_ps[64:128, fs],
                                lhsT=xt[64:128, li * 64 : (li + 1) * 64],
                                rhs=ut2_sb[64:128, :],
                                start=True,
                                stop=True,
                                tile_position=(64, 64),
                            )
                        else:
                            nc.tensor.matmul(
                                st2_ps[:, fs],
                                lhsT=xt[:, li * 128 : (li + 1) * 128],
                                rhs=ut2_sb[:],
                                start=True,
                                stop=True,
                            )
                    # --- st2 copy (cast to bf16), engine per cp_cycle ---
                    st2_sb = spool.tile([128, sb_len * 128], mybir.dt.bfloat16)
                    copy_ps(st2_sb[:], st2_ps[:])
                    # --- mm2 + out copy in batches of cpb pairs ---
                    off2 = 0
                    while off2 < sb_len:
                        chunk = min(cpb, sb_len - off2)
                        o_ps = ps2.tile([128, chunk * 256], mybir.dt.float32)
                        for p in range(chunk):
                            nc.tensor.matmul(
                                o_ps[:, p * 256 : (p + 1) * 256],
                                lhsT=st2_sb[:, (off2 + p) * 128 : (off2 + p + 1) * 128],
                                rhs=v2_sb[:],
                                start=True,
                                stop=True,
                            )
                        if direct:
                            p0 = og_base + off + off2
                            dst = out[:, p0 * 2 : (p0 + chunk) * 2, :]
                            nc.gpsimd.dma_start(
                                out=dst.rearrange("p g f -> p (g f)"),
                                in_=o_ps[:],
                            )
                        else:
                            oslice = ob[
                                :, (off + off2) * 256 : (off + off2 + chunk) * 256
                            ]
                            copy_ps(oslice, o_ps[:])
                        off2 += chunk
                    off += sb_len
            if not direct:
                dst = out[:, og_base * 2 : (og_base + og_len) * 2, :]
                eng = dma_out
                if CFG["out_alt"]:
                    eng = (
                        nc.sync
                        if (og_base // max(og_len, 1)) % 2 == 0
                        else nc.scalar
                    )
                elif CFG.get("out_alt2"):
                    eng = nc.sync if g_idx % 2 == 0 else nc.gpsimd
                eng.dma_start(
                    out=dst.rearrange("p g f -> p (g f)"), in_=ob[:]
                )
            og_base += og_len


_NC_CACHE = None


def _build_nc():
    nc = _make_nc()
    xin = nc.declare_dram_parameter(
        "xin", _xin_shape(), mybir.dt.bfloat16, isOutput=False
    )
    ut2 = nc.declare_dram_parameter(
        "ut2", [128, 128], mybir.dt.bfloat16, isOutput=False
    )
    v2 = nc.declare_dram_parameter(
        "v2", [128, 256], mybir.dt.bfloat16, isOutput=False
    )
    out = nc.declare_dram_parameter(
        "out", [128, IMGS, 128], _ODT[CFG["out_dtype"]][0], isOutput=True
    )
    with TileContext(nc) as tc:
        _emit_body(nc, tc, xin, ut2, v2, out)
    nc.compile()
    return nc


def build_nc_timed(iters: int):
    """Benchmark variant: internal DRAM I/O, body repeated `iters` times
    via a device-side loop, tiny external output for minimal transfer."""
    nc = _make_nc()
    dummy_in = nc.declare_dram_parameter(
        "dummy_in", [1, 4], mybir.dt.float32, isOutput=False
    )
    dummy_out = nc.declare_dram_parameter(
        "dummy_out", [1, 4], mybir.dt.float32, isOutput=True
    )
    xin = nc.dram_tensor("xin_i", _xin_shape(), mybir.dt.bfloat16)
    ut2 = nc.dram_tensor("ut2_i", [128, 128], mybir.dt.bfloat16)
    v2 = nc.dram_tensor("v2_i", [128, 256], mybir.dt.bfloat16)
    out = nc.dram_tensor("out_i", [128, IMGS, 128], _ODT[CFG["out_dtype"]][0])
    with TileContext(nc) as tc:
        if iters == 1:
            _emit_body(nc, tc, xin, ut2, v2, out)
        else:
            with tc.For_i(0, iters, 1):
                _emit_body(nc, tc, xin, ut2, v2, out)
        with tc.tile_pool(name="dummy", bufs=1) as dpool:
            dt_sb = dpool.tile([1, 4], mybir.dt.float32)
            nc.sync.dma_start(out=dt_sb[:], in_=dummy_in[:])
            nc.sync.dma_start(out=dummy_out[:], in_=dt_sb[:])
    nc.compile()
    return nc


def _host_pack(x_lowres, sparse_mask):
    """Fold mask into input and pack per-core block-diagonal pair slabs."""
    u = _upsample_mat()                      # [128, 64] float64
    ut = u.T.astype(np.float32)              # [64, 128]
    ut2_np = np.concatenate([ut, ut], axis=0).astype(BF16)      # [128, 128]
    v2_np = np.zeros((128, 256), dtype=BF16)                    # blockdiag(V, V)
    v2_np[0:64, 0:128] = ut.astype(BF16)
    v2_np[64:128, 128:256] = ut.astype(BF16)

    xm = (x_lowres.astype(np.float32) * sparse_mask.astype(np.float32)).astype(BF16)

    mode = CFG["mode"]
    in_maps = []
    for i in range(N_CORES):
        imgs = xm[i * B_PER_CORE : (i + 1) * B_PER_CORE].reshape(IMGS, H, W)
        if mode == "hpair":
            xpack = np.empty((64, PAIRS, 128), dtype=BF16)
            xpack[:, :, 0:64] = imgs[0::2].transpose(1, 0, 2)
            xpack[:, :, 64:128] = imgs[1::2].transpose(1, 0, 2)
        elif mode == "vpair":
            xpack = np.empty((128, PAIRS, 64), dtype=BF16)
            xpack[0:64] = imgs[0::2].transpose(1, 0, 2)
            xpack[64:128] = imgs[1::2].transpose(1, 0, 2)
        else:
            xpack = np.zeros((128, PAIRS, 128), dtype=BF16)
            xpack[0:64, :, 0:64] = imgs[0::2].transpose(1, 0, 2)
            xpack[64:128, :, 64:128] = imgs[1::2].transpose(1, 0, 2)
        in_maps.append({"xin": xpack, "ut2": ut2_np, "v2": v2_np})
    return in_maps


def kernel(x_lowres: np.ndarray, sparse_mask: np.ndarray) -> np.ndarray:
    global _NC_CACHE, LAST_RESULT
    x_lowres = np.asarray(x_lowres)
    sparse_mask = np.asarray(sparse_mask)
    assert x_lowres.shape == (B, C, H, W), x_lowres.shape

    in_maps = _host_pack(x_lowres, sparse_mask)

    if _NC_CACHE is None:
        _NC_CACHE = _build_nc()
    nc = _NC_CACHE

    trace = bool(os.environ.get("BASS_TRACE"))
    try:
        res = run_bass_kernel_spmd(nc, in_maps, list(range(N_CORES)), trace=trace)
    except ModuleNotFoundError:
        # Trace path needs the axon NTFF hook; absent in slim containers.
        os.environ["BASS_NEVER_TRACE"] = "1"
        res = run_bass_kernel_spmd(nc, in_maps, list(range(N_CORES)), trace=False)
    LAST_RESULT = res

    out = np.empty((B, C, H2, W2), dtype=np.float32)
    for i in range(N_CORES):
        dev = np.asarray(res.results[i]["out"])          # [128, IMGS, 128]
        out[i * B_PER_CORE : (i + 1) * B_PER_CORE] = (
            dev.transpose(1, 0, 2).reshape(B_PER_CORE, C, H2, W2)
            .astype(np.float32)
        )
    return out

